# revision 10
# baseline (speedup 1.0000x reference)
"""BERT self-attention (B=8, S=1024, D=1024, H=16, DH=64) on 8 Trainium2 cores.

Strategy: pure data-parallel over batch - each of the 8 cores runs the full
self-attention for one batch element. No collectives.

v3 design (393us v1 -> 304us v2 -> this): everything upstream of PSUM runs
in bf16 (SWDGE cast-DMAs on all loads), scores run as strictly ALTERNATED
head-pair matmuls (HW-measured: serial half-array K=64 MMs cost 435ns,
alternating row-groups 113ns/MM - concurrent quadrants), the context matmul
is natural-orientation N=65 with per-MM weight reload (HW-measured 61ns/MM
incl. LDWEIGHTS), and projections/attention are software-pipelined per
head-PAIR so the ACT engine's irreducible exp stream (~147us for the 16.8M
softmax elements) overlaps projection matmuls instead of serializing after
them.

Per-core layout:
  - X^T via 64 PE transposes (bf16, 1 cyc/row) from cast-loaded X tiles.
  - W col-blocks [1024,128] loaded per head-pair as [128, 8*128] bf16 tiles
    (SWDGE cast + 3D-AP DMA); Q^T/K^T computed in [128,512] PSUM chains,
    bias folded into the DVE PSUM->SBUF copy as a per-partition
    tensor_scalar_add (no rank-1 bias matmuls).
  - scores TRANSPOSED per head pair: S^T[k,q]; the two heads of a pair sit
    on partitions 0:64 / 64:128 of qT/kT, so their K=64 matmuls auto-derive
    tile_position (0,0)/(64,0) and run CONCURRENTLY in disjoint row-groups
    of the PE array when strictly alternated (~2x).
  - P^T = exp(scale*S^T + mask[k]) on ACT, bf16 out; mask is a
    per-partition bias column, so arbitrary masks are free.
  - context natural: ctx[q,0:64] + rowsum at col 64 in PSUM [128,65];
    lhsT = P^T q-block (fresh weights each MM), rhs = V 65-block with ones
    column emitting the softmax denominator; DVE reciprocal + per-partition
    tensor_scalar_mul normalize into bf16 staging; out DRAM tensor is bf16
    (host converts) so stores are 16 big [128,512] HWDGE DMAs.
  - steady-state block for pair p: scores(p) + exp(p) + ctx(p-2) +
    Q/K projections(p+1) interleaved at small-group granularity so the PE
    always has ready work while ACT paces the softmax.

Built on bacc.Bacc: its compile() legalizes sync waits (1 wait/instruction
hardware limit) via move_matmul_waits_to_ldweights + generate_event_semaphores.
"""

import numpy as np

import concourse.bass as bass
import concourse.bacc as bacc
import concourse.mybir as mybir
import concourse.tile as tile
from concourse.bass_utils import run_bass_kernel_spmd
from concourse.masks import make_identity

F32 = mybir.dt.float32
BF16 = mybir.dt.bfloat16

B, S, D, H = 8, 1024, 1024, 16
DH = D // H  # 64
P = 128
NT = S // P  # 8 tiles along any 1024 dim
NP = H // 2  # 8 head pairs
SCALE = 1.0 / float(np.sqrt(DH))
N_CORES = 8
VW = DH + 1  # 65: V block width per head (64 cols + ones col)


class Ctx:
    """Emission context: nc + dram handles + pools + persistent tiles."""

    pass


def _emit_consts(c):
    nc = c.nc
    c.mask_cols = c.cst.tile([P, NT], F32, name="mask_cols", tag="mask_cols")
    nc.sync.dma_start(out=c.mask_cols, in_=c.m_d.ap().rearrange("(g p) -> p g", p=P))
    c.bq_cols = c.cst.tile([P, NT], F32, name="bq_cols", tag="bq_cols")
    nc.sync.dma_start(out=c.bq_cols, in_=c.bq_d.ap().rearrange("(g p) -> p g", p=P))
    c.bk_cols = c.cst.tile([P, NT], F32, name="bk_cols", tag="bk_cols")
    nc.sync.dma_start(out=c.bk_cols, in_=c.bk_d.ap().rearrange("(g p) -> p g", p=P))
    c.bv_row = c.cst.tile([1, D], BF16, name="bv_row", tag="bv_row")
    nc.gpsimd.dma_start(out=c.bv_row, in_=c.bv_d.ap().unsqueeze(0))
    c.ones_row = c.cst.tile([1, P], BF16, name="ones_row", tag="ones_row")
    nc.vector.memset(c.ones_row, 1.0)
    # v_sb memsets: ones columns at h*65+64 survive the V copy.
    for st in range(NT):
        nc.vector.memset(c.v_sb[st], 1.0)


def _emit_w_dma(c, p):
    """Load Wq/Wk column-block p as [128, 8*128] bf16 tiles (cast DMA).

    tile[i%128, (i//128)*128 + j] = W[i, p*128 + j], so the it-th 128-col
    slice is the lhsT [i_part, j_cols] for contraction row-block it.
    """
    nc = c.nc
    for nm, w_d in (("q", c.wq_d), ("k", c.wk_d)):
        t = c.wqk_pool.tile([P, D], BF16, name=f"w{nm}{p}", tag="wqk")
        nc.gpsimd.dma_start(
            out=t.rearrange("p (it j) -> p it j", j=P),
            in_=w_d.ap()[:, p * P : (p + 1) * P].rearrange("(it p) j -> p it j", p=P),
        )
        c.w_tiles[(nm, p)] = t


def _emit_wv_dma(c):
    nc = c.nc
    c.wv_tiles = []
    for it in range(NT):
        t = c.stage_pool.tile([P, D], BF16, name=f"wv{it}", tag="stage")
        nc.gpsimd.dma_start(out=t, in_=c.wv_d.ap()[it * P : (it + 1) * P, :])
        c.wv_tiles.append(t)


def _emit_phase_a(c):
    """Cast-load X and build X^T (bf16) via PE transposes."""
    nc = c.nc
    xbs = []
    for st in range(NT):
        xb = c.stage_pool.tile([P, D], BF16, name=f"xb{st}", tag="stage")
        nc.gpsimd.dma_start(out=xb, in_=c.x_d.ap()[st * P : (st + 1) * P, :])
        xbs.append(xb)
    # W col-blocks for pairs 0/1 queue behind the X loads on the SWDGE ring.
    _emit_w_dma(c, 0)
    _emit_w_dma(c, 1)
    # 8 transposes packed per 1-bank PSUM super-tile: cross-engine handoff
    # (sem latency ~175ns) is paid once per st instead of once per transpose
    for st in range(NT):
        sup = c.ps_work.tile([P, S], BF16, name="trsup", tag="work")
        for it in range(NT):
            nc.tensor.transpose(
                sup[:, it * P : (it + 1) * P],
                xbs[st][:, it * P : (it + 1) * P],
                c.ident,
            )
        for it in range(NT):
            nc.vector.tensor_copy(
                c.xT[it][:, st * P : (st + 1) * P], sup[:, it * P : (it + 1) * P]
            )


def _gen_qk_proj(c, p):
    """8 groups: Q^T[p], K^T[p] in four [128,512] PSUM chains of 8 matmuls."""
    nc = c.nc
    dst_q = c.qkT_pool.tile([P, S], BF16, name=f"qT{p}", tag="qT")
    dst_k = c.qkT_pool.tile([P, S], BF16, name=f"kT{p}", tag="kT")
    c.qT[p], c.kT[p] = dst_q, dst_k
    groups = []
    for nm, dst, bcol in (("q", dst_q, c.bq_cols), ("k", dst_k, c.bk_cols)):
        for sc in range(2):
            for half in range(2):
                def g(nm=nm, dst=dst, bcol=bcol, sc=sc, half=half):
                    w = c.w_tiles[(nm, p)]
                    if half == 0:
                        ps = c.ps_work.tile([P, 512], F32, name="psp", tag="work")
                        c._proj_ps = ps
                    else:
                        ps = c._proj_ps
                    for it in range(4 * half, 4 * half + 4):
                        nc.tensor.matmul(
                            ps,
                            lhsT=w[:, it * P : (it + 1) * P],
                            rhs=c.xT[it][:, sc * 512 : (sc + 1) * 512],
                            start=(it == 0),
                            stop=(it == NT - 1),
                        )
                    if half == 1:
                        nc.vector.tensor_scalar_add(
                            dst[:, sc * 512 : (sc + 1) * 512], ps, bcol[:, p : p + 1]
                        )
                groups.append(g)
    return groups


def _emit_v_proj(c):
    """V natural [s, j] into 65-wide head blocks, bf16, + bias matmul.

    Full-width [128,1024] PSUM tiles from the sc pool, it-outer with both
    512-halves per weight load (double-pumped lhsT, half the LDWEIGHTS).
    """
    nc = c.nc
    for st in range(NT):
        ps = c.ps_sc.tile([P, S], F32, name="psv", tag="sc")
        for it in range(NT):
            for jc in range(2):
                nc.tensor.matmul(
                    ps[:, jc * 512 : (jc + 1) * 512],
                    lhsT=c.xT[it][:, st * P : (st + 1) * P],
                    rhs=c.wv_tiles[it][:, jc * 512 : (jc + 1) * 512],
                    start=(it == 0),
                    stop=False,
                )
        for jc in range(2):
            nc.tensor.matmul(
                ps[:, jc * 512 : (jc + 1) * 512],
                lhsT=c.ones_row[0:1, 0:P],
                rhs=c.bv_row[0:1, jc * 512 : (jc + 1) * 512],
                start=False,
                stop=True,
            )
        dst = c.v_sb[st].rearrange("p (g c) -> p g c", c=VW)[:, :, 0:DH]
        nc.vector.tensor_copy(dst, ps.rearrange("p (g c) -> p g c", c=DH))


def _gen_scores(c, p):
    """8 groups (one per kt): 4 score matmuls (2 heads row-tiled) + 2 exps."""
    nc = c.nc
    groups = []
    for kt in range(NT):
        def g(kt=kt):
            # strict row-group alternation: (A,B) pairs run concurrently in
            # disjoint quadrants (serial half-array MMs cost 2x - HW measured)
            pss = {}
            for hl, ro in ((0, 0), (1, DH)):
                pss[hl] = c.ps_sc.tile([P, S], F32, name="pss", tag="sc")
            for qc in range(2):
                for hl, ro in ((0, 0), (1, DH)):
                    nc.tensor.matmul(
                        pss[hl][:, qc * 512 : (qc + 1) * 512],
                        lhsT=c.kT[p][ro : ro + DH, kt * P : (kt + 1) * P],
                        rhs=c.qT[p][ro : ro + DH, qc * 512 : (qc + 1) * 512],
                        start=True,
                        stop=True,
                    )
            for hl in (0, 1):
                pt = c.pT_pool.tile([P, S], BF16, name="pT", tag="pT")
                nc.scalar.activation(
                    pt,
                    pss[hl],
                    mybir.ActivationFunctionType.Exp,
                    bias=c.mask_cols[:, kt : kt + 1],
                    scale=SCALE,
                )
                c.pT[(p, hl, kt)] = pt
        groups.append(g)
    return groups


def _gen_ctx(c, p):
    """8 groups: natural ctx for pair p; group s = both heads' qt=s chains.

    ctx[q, 0:64]+rowsum at col 64 via lhsT = P^T q-block (new weights every
    matmul: 61ns/MM HW-measured incl. LDWEIGHTS at N=65), rhs = V 65-block;
    normalize with reciprocal + per-partition scalar mul into staging.
    """
    nc = c.nc
    groups = []
    for s in range(8):
        def g(s=s):
            qt = s
            for hl in (0, 1):
                h = 2 * p + hl
                # 7 chains share a 1-bank [128, 455] super-tile: no PSUM WAR
                # between chains, one batched reciprocal per super-tile
                if c.ctx_slot == 0:
                    c.ctx_super = c.ps_ctx.tile(
                        [P, 7 * VW], F32, name="ctxsup", tag="ctx"
                    )
                    c.ctx_pend = []
                sl = c.ctx_slot
                ps = c.ctx_super[:, sl * VW : sl * VW + VW]
                for kt in range(NT):
                    nc.tensor.matmul(
                        ps,
                        lhsT=c.pT[(p, hl, kt)][:, qt * P : (qt + 1) * P],
                        rhs=c.v_sb[kt][:, h * VW : (h + 1) * VW],
                        start=(kt == 0),
                        stop=(kt == NT - 1),
                    )
                c.ctx_pend.append((h, qt, sl))
                c.ctx_slot += 1
                if c.ctx_slot == 7:
                    _flush_ctx(c)
        groups.append(g)
    return groups


def _flush_ctx(c):
    """Normalize the pending chains of the current ctx super-tile."""
    nc = c.nc
    if not getattr(c, "ctx_pend", None):
        return
    n = len(c.ctx_pend)
    r = c.small.tile([P, 7], F32, name="r", tag="r")
    denoms = c.ctx_super.rearrange("p (c w) -> p c w", w=VW)[:, 0:n, DH]
    nc.vector.reciprocal(r[:, 0:n], denoms)
    for h, qt, sl in c.ctx_pend:
        nc.vector.tensor_scalar_mul(
            c.out_sb[qt][:, h * DH : (h + 1) * DH],
            c.ctx_super[:, sl * VW : sl * VW + DH],
            r[:, sl : sl + 1],
        )
    c.ctx_pend = []
    c.ctx_slot = 0


def _emit_out_dma(c, half, engines):
    for qt in range(NT):
        eng = engines[qt % len(engines)]
        eng.dma_start(
            out=c.o_d.ap()[qt * P : (qt + 1) * P, half * 512 : (half + 1) * 512],
            in_=c.out_sb[qt][:, half * 512 : (half + 1) * 512],
        )


def emit_body(nc, dram, pools):
    c = Ctx()
    c.nc = nc
    (c.x_d, c.m_d, c.wq_d, c.bq_d, c.wk_d, c.bk_d, c.wv_d, c.bv_d, c.o_d) = dram
    (c.cst, c.stage_pool, c.wqk_pool, c.xT_pool, c.qkT_pool, c.v_pool,
     c.pT_pool, c.out_pool, c.small,
     c.ps_sc, c.ps_work, c.ps_ctx, c.ident) = pools

    c.w_tiles, c.qT, c.kT, c.pT = {}, {}, {}, {}
    c.ctx_slot, c.ctx_pend = 0, []
    c.xT = [
        c.xT_pool.tile([P, S], BF16, name=f"xT{it}", tag=f"xT{it}")
        for it in range(NT)
    ]
    c.v_sb = [
        c.v_pool.tile([P, H * VW], BF16, name=f"v{st}", tag=f"v{st}")
        for st in range(NT)
    ]
    c.out_sb = [
        c.out_pool.tile([P, S], BF16, name=f"o{qt}", tag=f"o{qt}")
        for qt in range(NT)
    ]

    _emit_consts(c)
    # ---- preamble: X^T, QK0-3, sc0-2, V (ACT digests sc0-2 during V/QK) ----
    _emit_phase_a(c)  # also issues W dmas for pairs 0,1
    for g in _gen_qk_proj(c, 0):
        g()
    for g in _gen_scores(c, 0):
        g()
    _emit_w_dma(c, 2)
    for g in _gen_qk_proj(c, 1):
        g()
    for g in _gen_scores(c, 1):
        g()
    _emit_wv_dma(c)
    _emit_w_dma(c, 3)
    _emit_v_proj(c)
    for g in _gen_qk_proj(c, 2):
        g()
    for g in _gen_scores(c, 2):
        g()
    _emit_w_dma(c, 4)
    for g in _gen_qk_proj(c, 3):
        g()
    for g in _gen_ctx(c, 0):
        g()

    # ---- steady-state blocks p = 3..7: sc(p) + ctx(p-2) + proj(p+1) ----
    for p in range(3, NP):
        if p + 2 < NP:
            _emit_w_dma(c, p + 2)
        if p == 6:
            # heads 0..7 (pairs 0..3, ctx done in block 5) cover out cols 0:512
            _flush_ctx(c)  # pair 3's tail chains may sit in a partial super
            _emit_out_dma(c, 0, [nc.sync])
        sc_g = _gen_scores(c, p)
        ctx_g = _gen_ctx(c, p - 2)
        proj_g = _gen_qk_proj(c, p + 1) if p + 1 < NP else [None] * 8
        for s in range(8):
            # sc first: ACT (the pacing engine) gets its tiles ASAP; the
            # sc slot-waits are pre-satisfied since ACT runs a step behind
            # (HW-measured 2284 vs 2783 ns/step for sc-last)
            sc_g[s]()
            ctx_g[s]()
            if proj_g[s] is not None:
                proj_g[s]()

    # ---- drain: ctx6, ctx7 + remaining output ----
    for g in _gen_ctx(c, NP - 2):
        g()
    for g in _gen_ctx(c, NP - 1):
        g()
    _flush_ctx(c)
    _emit_out_dma(c, 1, [nc.sync, nc.scalar])


def build_program(n_reps: int = 1, n_loop: int = 0) -> bass.Bass:
    nc = bacc.Bacc(trn_type="TRN2", target_bir_lowering=False, debug=False)

    x_d = nc.declare_dram_parameter("hidden_states", [S, D], F32, isOutput=False)
    m_d = nc.declare_dram_parameter("attention_mask", [S], F32, isOutput=False)
    wq_d = nc.declare_dram_parameter("Wq", [D, D], F32, isOutput=False)
    bq_d = nc.declare_dram_parameter("bq", [D], F32, isOutput=False)
    wk_d = nc.declare_dram_parameter("Wk", [D, D], F32, isOutput=False)
    bk_d = nc.declare_dram_parameter("bk", [D], F32, isOutput=False)
    wv_d = nc.declare_dram_parameter("Wv", [D, D], F32, isOutput=False)
    bv_d = nc.declare_dram_parameter("bv", [D], F32, isOutput=False)
    o_d = nc.declare_dram_parameter("out", [S, D], BF16, isOutput=True)
    dram = (x_d, m_d, wq_d, bq_d, wk_d, bk_d, wv_d, bv_d, o_d)

    with tile.TileContext(nc) as tc:
        with (
            tc.tile_pool(name="consts", bufs=1) as cst,
            tc.tile_pool(name="stage", bufs=8) as stage_pool,  # X then Wv tiles
            tc.tile_pool(name="wqk", bufs=4) as wqk_pool,
            tc.tile_pool(name="xT", bufs=1) as xT_pool,
            tc.tile_pool(name="qkT", bufs=3) as qkT_pool,
            tc.tile_pool(name="vsb", bufs=1) as v_pool,
            tc.tile_pool(name="pT", bufs=48) as pT_pool,
            tc.tile_pool(name="osb", bufs=1) as out_pool,
            tc.tile_pool(name="small", bufs=16) as small_pool,
            # PSUM is bank-granular (8 banks x 2KB): sc 2x2 + work 2x1
            # (proj chains / phase-A transpose super-tiles) + ctx 2x1 = 8
            tc.tile_pool(name="pssc", bufs=2, space="PSUM") as ps_sc,
            tc.tile_pool(name="pswork", bufs=2, space="PSUM") as ps_work,
            tc.tile_pool(name="psctx", bufs=2, space="PSUM") as ps_ctx,
        ):
            ident = cst.tile([P, P], BF16, name="ident", tag="ident")
            make_identity(nc, ident)
            pools = (cst, stage_pool, wqk_pool, xT_pool, qkT_pool, v_pool,
                     pT_pool, out_pool, small_pool,
                     ps_sc, ps_work, ps_ctx, ident)
            if n_loop:
                with tc.For_i(0, n_loop, 1):
                    emit_body(nc, dram, pools)
            else:
                for _ in range(n_reps):
                    emit_body(nc, dram, pools)
    nc.compile()
    return nc


_NC_CACHE = None


def _get_nc():
    global _NC_CACHE
    if _NC_CACHE is None:
        _NC_CACHE = build_program()
    return _NC_CACHE


def make_in_maps(hidden_states, attention_mask, Wq, bq, Wk, bk, Wv, bv):
    hs = np.ascontiguousarray(np.asarray(hidden_states, dtype=np.float32))
    am = np.ascontiguousarray(
        np.asarray(attention_mask, dtype=np.float32).reshape(B, S)
    )
    shared = {
        "Wq": np.ascontiguousarray(np.asarray(Wq, dtype=np.float32)),
        "bq": np.ascontiguousarray(np.asarray(bq, dtype=np.float32)),
        "Wk": np.ascontiguousarray(np.asarray(Wk, dtype=np.float32)),
        "bk": np.ascontiguousarray(np.asarray(bk, dtype=np.float32)),
        "Wv": np.ascontiguousarray(np.asarray(Wv, dtype=np.float32)),
        "bv": np.ascontiguousarray(np.asarray(bv, dtype=np.float32)),
    }
    return [
        {"hidden_states": hs[b], "attention_mask": am[b], **shared}
        for b in range(B)
    ]


def kernel(hidden_states, attention_mask, Wq, bq, Wk, bk, Wv, bv):
    nc = _get_nc()
    in_maps = make_in_maps(hidden_states, attention_mask, Wq, bq, Wk, bk, Wv, bv)
    res = run_bass_kernel_spmd(nc, in_maps, list(range(N_CORES))).results
    out = np.stack([np.asarray(res[b]["out"], dtype=np.float32) for b in range(B)])
    return out


# revision 12
# speedup vs baseline: 1.0461x; 1.0461x over previous
"""BERT self-attention (B=8, S=1024, D=1024, H=16, DH=64) on 8 Trainium2 cores.

Strategy: pure data-parallel over batch - each of the 8 cores runs the full
self-attention for one batch element. No collectives.

v3 design (393us v1 -> 304us v2 -> this): everything upstream of PSUM runs
in bf16 (SWDGE cast-DMAs on all loads), scores run as strictly ALTERNATED
head-pair matmuls (HW-measured: serial half-array K=64 MMs cost 435ns,
alternating row-groups 113ns/MM - concurrent quadrants), the context matmul
is natural-orientation N=65 with per-MM weight reload (HW-measured 61ns/MM
incl. LDWEIGHTS), and projections/attention are software-pipelined per
head-PAIR so the ACT engine's irreducible exp stream (~147us for the 16.8M
softmax elements) overlaps projection matmuls instead of serializing after
them.

Per-core layout:
  - X^T via 64 PE transposes (bf16, 1 cyc/row) from cast-loaded X tiles.
  - W col-blocks [1024,128] loaded per head-pair as [128, 8*128] bf16 tiles
    (SWDGE cast + 3D-AP DMA); Q^T/K^T computed in [128,512] PSUM chains,
    bias folded into the DVE PSUM->SBUF copy as a per-partition
    tensor_scalar_add (no rank-1 bias matmuls).
  - scores TRANSPOSED per head pair: S^T[k,q]; the two heads of a pair sit
    on partitions 0:64 / 64:128 of qT/kT, so their K=64 matmuls auto-derive
    tile_position (0,0)/(64,0) and run CONCURRENTLY in disjoint row-groups
    of the PE array when strictly alternated (~2x).
  - P^T = exp(scale*S^T + mask[k]) on ACT, bf16 out; mask is a
    per-partition bias column, so arbitrary masks are free.
  - context natural: ctx[q,0:64] + rowsum at col 64 in PSUM [128,65];
    lhsT = P^T q-block (fresh weights each MM), rhs = V 65-block with ones
    column emitting the softmax denominator; DVE reciprocal + per-partition
    tensor_scalar_mul normalize into bf16 staging; out DRAM tensor is bf16
    (host converts) so stores are 16 big [128,512] HWDGE DMAs.
  - steady-state block for pair p: scores(p) + exp(p) + ctx(p-2) +
    Q/K projections(p+1) interleaved at small-group granularity so the PE
    always has ready work while ACT paces the softmax.

Built on bacc.Bacc: its compile() legalizes sync waits (1 wait/instruction
hardware limit) via move_matmul_waits_to_ldweights + generate_event_semaphores.
"""

import numpy as np

import concourse.bass as bass
import concourse.bacc as bacc
import concourse.mybir as mybir
import concourse.tile as tile
from concourse.bass_utils import run_bass_kernel_spmd
from concourse.masks import make_identity

F32 = mybir.dt.float32
BF16 = mybir.dt.bfloat16

B, S, D, H = 8, 1024, 1024, 16
DH = D // H  # 64
P = 128
NT = S // P  # 8 tiles along any 1024 dim
NP = H // 2  # 8 head pairs
SCALE = 1.0 / float(np.sqrt(DH))
N_CORES = 8
VW = DH + 1  # 65: V block width per head (64 cols + ones col)


class Ctx:
    """Emission context: nc + dram handles + pools + persistent tiles."""

    pass


def _emit_consts(c):
    nc = c.nc
    c.mask_cols = c.cst.tile([P, NT], F32, name="mask_cols", tag="mask_cols")
    nc.sync.dma_start(out=c.mask_cols, in_=c.m_d.ap().rearrange("(g p) -> p g", p=P))
    c.bq_cols = c.cst.tile([P, NT], F32, name="bq_cols", tag="bq_cols")
    nc.sync.dma_start(out=c.bq_cols, in_=c.bq_d.ap().rearrange("(g p) -> p g", p=P))
    c.bk_cols = c.cst.tile([P, NT], F32, name="bk_cols", tag="bk_cols")
    nc.sync.dma_start(out=c.bk_cols, in_=c.bk_d.ap().rearrange("(g p) -> p g", p=P))
    c.bv_row = c.cst.tile([1, D], BF16, name="bv_row", tag="bv_row")
    nc.gpsimd.dma_start(out=c.bv_row, in_=c.bv_d.ap().unsqueeze(0))
    c.ones_row = c.cst.tile([1, P], BF16, name="ones_row", tag="ones_row")
    nc.vector.memset(c.ones_row, 1.0)
    # v_sb memsets: ones columns at h*65+64 survive the V copy.
    for st in range(NT):
        nc.vector.memset(c.v_sb[st], 1.0)


def _emit_w_dma(c, p):
    """Load Wq/Wk column-block p as [128, 8*128] bf16 tiles (cast DMA).

    tile[i%128, (i//128)*128 + j] = W[i, p*128 + j], so the it-th 128-col
    slice is the lhsT [i_part, j_cols] for contraction row-block it.
    """
    nc = c.nc
    for nm, w_d in (("q", c.wq_d), ("k", c.wk_d)):
        t = c.wqk_pool.tile([P, D], BF16, name=f"w{nm}{p}", tag="wqk")
        nc.gpsimd.dma_start(
            out=t.rearrange("p (it j) -> p it j", j=P),
            in_=w_d.ap()[:, p * P : (p + 1) * P].rearrange("(it p) j -> p it j", p=P),
        )
        c.w_tiles[(nm, p)] = t


def _emit_wv_dma(c):
    nc = c.nc
    c.wv_tiles = []
    for it in range(NT):
        t = c.stage_pool.tile([P, D], BF16, name=f"wv{it}", tag="stage")
        nc.gpsimd.dma_start(out=t, in_=c.wv_d.ap()[it * P : (it + 1) * P, :])
        c.wv_tiles.append(t)


def _emit_phase_a(c):
    """Cast-load X and build X^T (bf16) via PE transposes."""
    nc = c.nc
    xbs = []
    for st in range(NT):
        xb = c.stage_pool.tile([P, D], BF16, name=f"xb{st}", tag="stage")
        nc.gpsimd.dma_start(out=xb, in_=c.x_d.ap()[st * P : (st + 1) * P, :])
        xbs.append(xb)
    # W col-blocks for pairs 0/1 queue behind the X loads on the SWDGE ring.
    _emit_w_dma(c, 0)
    _emit_w_dma(c, 1)
    # 8 transposes packed per 1-bank PSUM super-tile: cross-engine handoff
    # (sem latency ~175ns) is paid once per st instead of once per transpose
    for st in range(NT):
        sup = c.ps_work.tile([P, S], BF16, name="trsup", tag="work")
        for it in range(NT):
            nc.tensor.transpose(
                sup[:, it * P : (it + 1) * P],
                xbs[st][:, it * P : (it + 1) * P],
                c.ident,
            )
        for it in range(NT):
            nc.vector.tensor_copy(
                c.xT[it][:, st * P : (st + 1) * P], sup[:, it * P : (it + 1) * P]
            )


def _gen_qk_proj(c, p):
    """8 groups: Q^T[p], K^T[p] in four [128,512] PSUM chains of 8 matmuls."""
    nc = c.nc
    dst_q = c.qkT_pool.tile([P, S], BF16, name=f"qT{p}", tag="qT")
    dst_k = c.qkT_pool.tile([P, S], BF16, name=f"kT{p}", tag="kT")
    c.qT[p], c.kT[p] = dst_q, dst_k
    groups = []
    for nm, dst, bcol in (("q", dst_q, c.bq_cols), ("k", dst_k, c.bk_cols)):
        for sc in range(2):
            for half in range(2):
                def g(nm=nm, dst=dst, bcol=bcol, sc=sc, half=half):
                    w = c.w_tiles[(nm, p)]
                    if half == 0:
                        ps = c.ps_work.tile([P, 512], F32, name="psp", tag="work")
                        c._proj_ps = ps
                    else:
                        ps = c._proj_ps
                    for it in range(4 * half, 4 * half + 4):
                        nc.tensor.matmul(
                            ps,
                            lhsT=w[:, it * P : (it + 1) * P],
                            rhs=c.xT[it][:, sc * 512 : (sc + 1) * 512],
                            start=(it == 0),
                            stop=(it == NT - 1),
                        )
                    if half == 1:
                        nc.vector.tensor_scalar_add(
                            dst[:, sc * 512 : (sc + 1) * 512], ps, bcol[:, p : p + 1]
                        )
                groups.append(g)
    return groups


def _emit_v_proj(c):
    """V natural [s, j] into 65-wide head blocks, bf16, + bias matmul.

    Full-width [128,1024] PSUM tiles from the sc pool, it-outer with both
    512-halves per weight load (double-pumped lhsT, half the LDWEIGHTS).
    """
    nc = c.nc
    for st in range(NT):
        ps = c.ps_sc.tile([P, S], F32, name="psv", tag="sc")
        for it in range(NT):
            for jc in range(2):
                nc.tensor.matmul(
                    ps[:, jc * 512 : (jc + 1) * 512],
                    lhsT=c.xT[it][:, st * P : (st + 1) * P],
                    rhs=c.wv_tiles[it][:, jc * 512 : (jc + 1) * 512],
                    start=(it == 0),
                    stop=False,
                )
        for jc in range(2):
            nc.tensor.matmul(
                ps[:, jc * 512 : (jc + 1) * 512],
                lhsT=c.ones_row[0:1, 0:P],
                rhs=c.bv_row[0:1, jc * 512 : (jc + 1) * 512],
                start=False,
                stop=True,
            )
        dst = c.v_sb[st].rearrange("p (g c) -> p g c", c=VW)[:, :, 0:DH]
        nc.vector.tensor_copy(dst, ps.rearrange("p (g c) -> p g c", c=DH))


def _gen_scores(c, p):
    """8 groups (one per kt): 4 score matmuls (2 heads row-tiled) + 2 exps."""
    nc = c.nc
    groups = []
    for kt in range(NT):
        def g(kt=kt):
            # strict row-group alternation: (A,B) pairs run concurrently in
            # disjoint quadrants (serial half-array MMs cost 2x - HW measured)
            pss = {}
            for hl, ro in ((0, 0), (1, DH)):
                pss[hl] = c.ps_sc.tile([P, S], F32, name="pss", tag="sc")
            for qc in range(2):
                for hl, ro in ((0, 0), (1, DH)):
                    nc.tensor.matmul(
                        pss[hl][:, qc * 512 : (qc + 1) * 512],
                        lhsT=c.kT[p][ro : ro + DH, kt * P : (kt + 1) * P],
                        rhs=c.qT[p][ro : ro + DH, qc * 512 : (qc + 1) * 512],
                        start=True,
                        stop=True,
                    )
            for hl in (0, 1):
                pt = c.pT_pool.tile([P, S], BF16, name="pT", tag="pT")
                nc.scalar.activation(
                    pt,
                    pss[hl],
                    mybir.ActivationFunctionType.Exp,
                    bias=c.mask_cols[:, kt : kt + 1],
                    scale=SCALE,
                )
                c.pT[(p, hl, kt)] = pt
        groups.append(g)
    return groups


def _gen_ctx(c, p):
    """8 groups: natural ctx for pair p; group s = both heads' qt=s chains.

    ctx[q, 0:64]+rowsum at col 64 via lhsT = P^T q-block (new weights every
    matmul: 61ns/MM HW-measured incl. LDWEIGHTS at N=65), rhs = V 65-block;
    normalize with reciprocal + per-partition scalar mul into staging.
    """
    nc = c.nc
    groups = []
    for s in range(8):
        def g(s=s):
            qt = s
            for hl in (0, 1):
                h = 2 * p + hl
                # 7 chains share a 1-bank [128, 455] super-tile: no PSUM WAR
                # between chains, one batched reciprocal per super-tile
                if c.ctx_slot == 0:
                    c.ctx_super = c.ps_ctx.tile(
                        [P, 7 * VW], F32, name="ctxsup", tag="ctx"
                    )
                    c.ctx_pend = []
                sl = c.ctx_slot
                ps = c.ctx_super[:, sl * VW : sl * VW + VW]
                for kt in range(NT):
                    nc.tensor.matmul(
                        ps,
                        lhsT=c.pT[(p, hl, kt)][:, qt * P : (qt + 1) * P],
                        rhs=c.v_sb[kt][:, h * VW : (h + 1) * VW],
                        start=(kt == 0),
                        stop=(kt == NT - 1),
                    )
                c.ctx_pend.append((h, qt, sl))
                c.ctx_slot += 1
                if c.ctx_slot == 7:
                    _flush_ctx(c)
        groups.append(g)
    return groups


def _flush_ctx(c):
    """Normalize the pending chains of the current ctx super-tile."""
    nc = c.nc
    if not getattr(c, "ctx_pend", None):
        return
    n = len(c.ctx_pend)
    r = c.small.tile([P, 7], F32, name="r", tag="r")
    denoms = c.ctx_super.rearrange("p (c w) -> p c w", w=VW)[:, 0:n, DH]
    nc.vector.reciprocal(r[:, 0:n], denoms)
    for h, qt, sl in c.ctx_pend:
        nc.vector.tensor_scalar_mul(
            c.out_sb[qt][:, h * DH : (h + 1) * DH],
            c.ctx_super[:, sl * VW : sl * VW + DH],
            r[:, sl : sl + 1],
        )
    c.ctx_pend = []
    c.ctx_slot = 0


def _emit_out_dma(c, half, engines):
    for qt in range(NT):
        eng = engines[qt % len(engines)]
        eng.dma_start(
            out=c.o_d.ap()[qt * P : (qt + 1) * P, half * 512 : (half + 1) * 512],
            in_=c.out_sb[qt][:, half * 512 : (half + 1) * 512],
        )


def emit_body(nc, dram, pools):
    c = Ctx()
    c.nc = nc
    (c.x_d, c.m_d, c.wq_d, c.bq_d, c.wk_d, c.bk_d, c.wv_d, c.bv_d, c.o_d) = dram
    (c.cst, c.stage_pool, c.wqk_pool, c.xT_pool, c.qkT_pool, c.v_pool,
     c.pT_pool, c.out_pool, c.small,
     c.ps_sc, c.ps_work, c.ps_ctx, c.ident) = pools

    c.w_tiles, c.qT, c.kT, c.pT = {}, {}, {}, {}
    c.ctx_slot, c.ctx_pend = 0, []
    c.xT = [
        c.xT_pool.tile([P, S], BF16, name=f"xT{it}", tag=f"xT{it}")
        for it in range(NT)
    ]
    c.v_sb = [
        c.v_pool.tile([P, H * VW], BF16, name=f"v{st}", tag=f"v{st}")
        for st in range(NT)
    ]
    c.out_sb = [
        c.out_pool.tile([P, S], BF16, name=f"o{qt}", tag=f"o{qt}")
        for qt in range(NT)
    ]

    _emit_consts(c)
    # ---- preamble: X^T, QK0-3, sc0-2, V (ACT digests sc0-2 during V/QK) ----
    _emit_phase_a(c)  # also issues W dmas for pairs 0,1
    for g in _gen_qk_proj(c, 0):
        g()
    for g in _gen_scores(c, 0):
        g()
    _emit_w_dma(c, 2)
    for g in _gen_qk_proj(c, 1):
        g()
    for g in _gen_scores(c, 1):
        g()
    _emit_wv_dma(c)
    _emit_w_dma(c, 3)
    _emit_v_proj(c)
    for g in _gen_qk_proj(c, 2):
        g()
    for g in _gen_scores(c, 2):
        g()
    _emit_w_dma(c, 4)
    for g in _gen_qk_proj(c, 3):
        g()
    for g in _gen_ctx(c, 0):
        g()

    # ---- steady-state blocks p = 3..7: sc(p) + ctx(p-2) + proj(p+1) ----
    for p in range(3, NP):
        if p + 2 < NP:
            _emit_w_dma(c, p + 2)
        if p == 6:
            # heads 0..7 (pairs 0..3, ctx done in block 5) cover out cols 0:512
            _flush_ctx(c)  # pair 3's tail chains may sit in a partial super
            _emit_out_dma(c, 0, [nc.sync])
        sc_g = _gen_scores(c, p)
        ctx_g = _gen_ctx(c, p - 2)
        proj_g = _gen_qk_proj(c, p + 1) if p + 1 < NP else [None] * 8
        # block 7 has no proj work left: take ctx6 as well, overlapping it
        # with ACT's final exp pair instead of serializing it in the drain
        ctx2_g = _gen_ctx(c, NP - 2) if p == NP - 1 else [None] * 8
        for s in range(8):
            # sc first: ACT (the pacer) gets tiles ASAP; slot-waits are
            # pre-satisfied since ACT trails (mb4: 2438 vs 2804 ns/step)
            sc_g[s]()
            ctx_g[s]()
            if proj_g[s] is not None:
                proj_g[s]()
            if ctx2_g[s] is not None:
                ctx2_g[s]()

    # ---- drain: ctx7 + remaining output ----
    for g in _gen_ctx(c, NP - 1):
        g()
    _flush_ctx(c)
    _emit_out_dma(c, 1, [nc.sync, nc.scalar])


def build_program(n_reps: int = 1, n_loop: int = 0) -> bass.Bass:
    nc = bacc.Bacc(trn_type="TRN2", target_bir_lowering=False, debug=False)

    x_d = nc.declare_dram_parameter("hidden_states", [S, D], F32, isOutput=False)
    m_d = nc.declare_dram_parameter("attention_mask", [S], F32, isOutput=False)
    wq_d = nc.declare_dram_parameter("Wq", [D, D], F32, isOutput=False)
    bq_d = nc.declare_dram_parameter("bq", [D], F32, isOutput=False)
    wk_d = nc.declare_dram_parameter("Wk", [D, D], F32, isOutput=False)
    bk_d = nc.declare_dram_parameter("bk", [D], F32, isOutput=False)
    wv_d = nc.declare_dram_parameter("Wv", [D, D], F32, isOutput=False)
    bv_d = nc.declare_dram_parameter("bv", [D], F32, isOutput=False)
    o_d = nc.declare_dram_parameter("out", [S, D], BF16, isOutput=True)
    dram = (x_d, m_d, wq_d, bq_d, wk_d, bk_d, wv_d, bv_d, o_d)

    with tile.TileContext(nc) as tc:
        with (
            tc.tile_pool(name="consts", bufs=1) as cst,
            tc.tile_pool(name="stage", bufs=8) as stage_pool,  # X then Wv tiles
            tc.tile_pool(name="wqk", bufs=4) as wqk_pool,
            tc.tile_pool(name="xT", bufs=1) as xT_pool,
            tc.tile_pool(name="qkT", bufs=3) as qkT_pool,
            tc.tile_pool(name="vsb", bufs=1) as v_pool,
            tc.tile_pool(name="pT", bufs=48) as pT_pool,
            tc.tile_pool(name="osb", bufs=1) as out_pool,
            tc.tile_pool(name="small", bufs=16) as small_pool,
            # PSUM is bank-granular (8 banks x 2KB): sc 2x2 + work 2x1
            # (proj chains / phase-A transpose super-tiles) + ctx 2x1 = 8
            tc.tile_pool(name="pssc", bufs=2, space="PSUM") as ps_sc,
            tc.tile_pool(name="pswork", bufs=2, space="PSUM") as ps_work,
            tc.tile_pool(name="psctx", bufs=2, space="PSUM") as ps_ctx,
        ):
            ident = cst.tile([P, P], BF16, name="ident", tag="ident")
            make_identity(nc, ident)
            pools = (cst, stage_pool, wqk_pool, xT_pool, qkT_pool, v_pool,
                     pT_pool, out_pool, small_pool,
                     ps_sc, ps_work, ps_ctx, ident)
            if n_loop:
                with tc.For_i(0, n_loop, 1):
                    emit_body(nc, dram, pools)
            else:
                for _ in range(n_reps):
                    emit_body(nc, dram, pools)
    nc.compile()
    return nc


_NC_CACHE = None


def _get_nc():
    global _NC_CACHE
    if _NC_CACHE is None:
        _NC_CACHE = build_program()
    return _NC_CACHE


def make_in_maps(hidden_states, attention_mask, Wq, bq, Wk, bk, Wv, bv):
    hs = np.ascontiguousarray(np.asarray(hidden_states, dtype=np.float32))
    am = np.ascontiguousarray(
        np.asarray(attention_mask, dtype=np.float32).reshape(B, S)
    )
    shared = {
        "Wq": np.ascontiguousarray(np.asarray(Wq, dtype=np.float32)),
        "bq": np.ascontiguousarray(np.asarray(bq, dtype=np.float32)),
        "Wk": np.ascontiguousarray(np.asarray(Wk, dtype=np.float32)),
        "bk": np.ascontiguousarray(np.asarray(bk, dtype=np.float32)),
        "Wv": np.ascontiguousarray(np.asarray(Wv, dtype=np.float32)),
        "bv": np.ascontiguousarray(np.asarray(bv, dtype=np.float32)),
    }
    return [
        {"hidden_states": hs[b], "attention_mask": am[b], **shared}
        for b in range(B)
    ]


def kernel(hidden_states, attention_mask, Wq, bq, Wk, bk, Wv, bv):
    nc = _get_nc()
    in_maps = make_in_maps(hidden_states, attention_mask, Wq, bq, Wk, bk, Wv, bv)
    res = run_bass_kernel_spmd(nc, in_maps, list(range(N_CORES))).results
    out = np.stack([np.asarray(res[b]["out"], dtype=np.float32) for b in range(B)])
    return out
